# revision 4
# baseline (speedup 1.0000x reference)
"""Trainium2 Bass kernel for nn_Encoder_57380763074770.

GRU-cell encoder over 64 independent "steps":
  xi  = concat(x[64,17], ip_emb[ip].reshape(64,8), port_emb[port].reshape(64,8))
  gi  = xi @ W_ih.T + b_ih            # [64, 384]
  gh  = h0 @ W_hh.T + b_hh            # [384], h0 = hidden[0,0]
  r   = sigmoid(gi_r + gh_r); z = sigmoid(gi_z + gh_z)
  n   = tanh(gi_n + r * gh_n)
  out = (1-z)*n + z*h0                # [64, 128]

Sharding: H=128 hidden columns split 8 ways -> each core owns 16 columns of
every gate (48 rows of W_ih/W_hh), computes out[:, 16c:16c+16].  Embedding
tables + x + indices are replicated; gathers run on-device via indirect DMA
so only the addressed table rows are read from HBM.
"""

import numpy as np

import concourse.bacc as bacc
import concourse.bass as bass
import concourse.mybir as mybir
import concourse.tile as tile
from concourse.bass_utils import run_bass_kernel_spmd

STEPS = 64
H = 128
N_CORES = 8
HS = H // N_CORES       # hidden cols per core = 16
G3 = 3 * HS             # gate rows per core = 48

F32 = mybir.dt.float32
I32 = mybir.dt.int32


def build_nc():
    nc = bacc.Bacc(None)

    x_d = nc.declare_dram_parameter("x", [STEPS, 17], F32, isOutput=False)
    ipi_d = nc.declare_dram_parameter("ipi", [STEPS, 8], I32, isOutput=False)
    pti_d = nc.declare_dram_parameter("pti", [STEPS, 2], I32, isOutput=False)
    h_d = nc.declare_dram_parameter("h", [H, 1], F32, isOutput=False)
    ipe_d = nc.declare_dram_parameter("ip_emb", [256, 1], F32, isOutput=False)
    pte_d = nc.declare_dram_parameter("port_emb", [70000, 4], F32, isOutput=False)
    wih_d = nc.declare_dram_parameter("wih", [G3, 33], F32, isOutput=False)
    whh_d = nc.declare_dram_parameter("whh", [G3, H], F32, isOutput=False)
    brow_d = nc.declare_dram_parameter("brow", [1, G3], F32, isOutput=False)
    bhhn_d = nc.declare_dram_parameter("bhhn", [1, HS], F32, isOutput=False)
    h0c_d = nc.declare_dram_parameter("h0c", [1, HS], F32, isOutput=False)
    ones_d = nc.declare_dram_parameter("ones", [1, STEPS], F32, isOutput=False)
    ident_d = nc.declare_dram_parameter("ident", [STEPS, STEPS], F32, isOutput=False)
    out_d = nc.declare_dram_parameter("out", [STEPS, HS], F32, isOutput=True)

    with tile.TileContext(nc) as tc:
        with (
            tc.tile_pool(name="sb", bufs=1) as sb,
            tc.tile_pool(name="ps", bufs=1, space="PSUM") as ps,
        ):
            xi = sb.tile([STEPS, 33], F32)
            ipi_t = sb.tile([STEPS, 8], I32)
            pti_t = sb.tile([STEPS, 2], I32)
            hcol = sb.tile([H, 1], F32)
            wih_s = sb.tile([G3, 33], F32)
            whh_s = sb.tile([G3, H], F32)
            brow = sb.tile([1, G3], F32)
            bhhn = sb.tile([1, HS], F32)
            h0c_s = sb.tile([1, HS], F32)
            ones = sb.tile([1, STEPS], F32)
            ident = sb.tile([STEPS, STEPS], F32)
            xiT = sb.tile([33, STEPS], F32)
            wihT = sb.tile([33, G3], F32)
            whhT = sb.tile([H, G3], F32)

            # loads (HWDGE)
            nc.sync.dma_start(out=ipi_t[:], in_=ipi_d[:, :])
            nc.sync.dma_start(out=pti_t[:], in_=pti_d[:, :])
            nc.sync.dma_start(out=wih_s[:], in_=wih_d[:, :])
            nc.sync.dma_start(out=whh_s[:], in_=whh_d[:, :])
            nc.sync.dma_start(out=ident[:], in_=ident_d[:, :])
            nc.sync.dma_start(out=xi[:, 0:17], in_=x_d[:, :])
            nc.sync.dma_start(out=hcol[:], in_=h_d[:, :])
            nc.sync.dma_start(out=brow[:], in_=brow_d[:, :])
            nc.sync.dma_start(out=bhhn[:], in_=bhhn_d[:, :])
            nc.sync.dma_start(out=h0c_s[:], in_=h0c_d[:, :])
            nc.sync.dma_start(out=ones[:], in_=ones_d[:, :])

            # embedding gathers: only the addressed table rows move from HBM.
            # HW indirect DMA takes ONE index per partition (contiguous read),
            # so gather column-by-column.
            for k in range(8):
                nc.gpsimd.indirect_dma_start(
                    out=xi[:, 17 + k : 18 + k],
                    out_offset=None,
                    in_=ipe_d[:, :],
                    in_offset=bass.IndirectOffsetOnAxis(ap=ipi_t[:, k : k + 1], axis=0),
                )
            for k in range(2):
                nc.gpsimd.indirect_dma_start(
                    out=xi[:, 25 + 4 * k : 29 + 4 * k],
                    out_offset=None,
                    in_=pte_d[:, :],
                    in_offset=bass.IndirectOffsetOnAxis(ap=pti_t[:, k : k + 1], axis=0),
                )

            # on-chip transposes (PE) into contraction-major layout
            p_wihT = ps.tile([33, G3], F32, space="PSUM")
            nc.tensor.transpose(out=p_wihT[:], in_=wih_s[:], identity=ident[:G3, :G3])
            nc.scalar.copy(out=wihT[:], in_=p_wihT[:])

            p_whhT = ps.tile([H, G3], F32, space="PSUM")
            nc.tensor.transpose(out=p_whhT[:], in_=whh_s[:], identity=ident[:G3, :G3])
            nc.scalar.copy(out=whhT[:], in_=p_whhT[:])

            p_xiT = ps.tile([33, STEPS], F32, space="PSUM")
            nc.tensor.transpose(out=p_xiT[:], in_=xi[:], identity=ident[:])
            nc.scalar.copy(out=xiT[:], in_=p_xiT[:])

            hcol_b = hcol[:, 0:1].to_broadcast([H, STEPS])

            # G = xi @ WihT + h0 @ WhhT + (b_ih + b_hh), all 48 gate cols
            G = ps.tile([STEPS, G3], F32, space="PSUM")
            nc.tensor.matmul(out=G[:], lhsT=xiT[:], rhs=wihT[:], start=True, stop=False)
            nc.tensor.matmul(out=G[:], lhsT=hcol_b, rhs=whhT[:], start=False, stop=False)
            nc.tensor.matmul(out=G[:], lhsT=ones[:], rhs=brow[:], start=False, stop=True)

            # hn = h0 @ WhhT_n + b_hh_n, broadcast to all 64 rows
            HN = ps.tile([STEPS, HS], F32, space="PSUM")
            nc.tensor.matmul(
                out=HN[:], lhsT=hcol_b, rhs=whhT[:, 2 * HS : 3 * HS], start=True, stop=False
            )
            nc.tensor.matmul(out=HN[:], lhsT=ones[:], rhs=bhhn[:], start=False, stop=True)

            # h0 broadcast to all rows (for the z*h0 term)
            H0B = ps.tile([STEPS, HS], F32, space="PSUM")
            nc.tensor.matmul(out=H0B[:], lhsT=ones[:], rhs=h0c_s[:], start=True, stop=True)

            sig = mybir.ActivationFunctionType.Sigmoid
            tanh = mybir.ActivationFunctionType.Tanh

            r = sb.tile([STEPS, HS], F32)
            z = sb.tile([STEPS, HS], F32)
            nc.scalar.activation(r[:], G[:, 0:HS], sig)
            nc.scalar.activation(z[:], G[:, HS : 2 * HS], sig)

            t1 = sb.tile([STEPS, HS], F32)
            nc.vector.tensor_mul(out=t1[:], in0=r[:], in1=HN[:])          # r*hn
            t2 = sb.tile([STEPS, HS], F32)
            nc.vector.tensor_sub(out=t2[:], in0=t1[:], in1=HN[:])         # (r-1)*hn
            u = sb.tile([STEPS, HS], F32)
            nc.vector.tensor_add(out=u[:], in0=G[:, 2 * HS : 3 * HS], in1=t2[:])
            n = sb.tile([STEPS, HS], F32)
            nc.scalar.activation(n[:], u[:], tanh)                        # n gate

            d = sb.tile([STEPS, HS], F32)
            nc.vector.tensor_sub(out=d[:], in0=H0B[:], in1=n[:])          # h0-n
            e = sb.tile([STEPS, HS], F32)
            nc.vector.tensor_mul(out=e[:], in0=z[:], in1=d[:])            # z*(h0-n)
            o = sb.tile([STEPS, HS], F32)
            nc.vector.tensor_add(out=o[:], in0=n[:], in1=e[:])            # n + z*(h0-n)

            nc.sync.dma_start(out=out_d[:, :], in_=o[:])

    nc.finalize()
    return nc


def make_in_maps(inputs):
    x = np.asarray(inputs["x"], dtype=np.float32)
    ipi = np.asarray(inputs["ip"], dtype=np.int32)
    pti = np.asarray(inputs["port"], dtype=np.int32)
    hid = np.asarray(inputs["hidden"], dtype=np.float32).reshape(H)
    ip_emb = np.ascontiguousarray(np.asarray(inputs["ip_emb"], dtype=np.float32))
    port_emb = np.ascontiguousarray(np.asarray(inputs["port_emb"], dtype=np.float32))
    W_ih = np.asarray(inputs["W_ih"], dtype=np.float32)
    W_hh = np.asarray(inputs["W_hh"], dtype=np.float32)
    b = np.asarray(inputs["b_ih"], dtype=np.float32) + np.asarray(
        inputs["b_hh"], dtype=np.float32
    )
    b_hh = np.asarray(inputs["b_hh"], dtype=np.float32)
    ones = np.ones((1, STEPS), dtype=np.float32)
    ident = np.eye(STEPS, dtype=np.float32)

    in_maps = []
    for c in range(N_CORES):
        sl = np.arange(c * HS, (c + 1) * HS)
        rows = np.concatenate([sl, H + sl, 2 * H + sl])
        in_maps.append(
            {
                "x": x,
                "ipi": ipi,
                "pti": pti,
                "h": hid.reshape(H, 1),
                "ip_emb": ip_emb,
                "port_emb": port_emb,
                "wih": np.ascontiguousarray(W_ih[rows]),
                "whh": np.ascontiguousarray(W_hh[rows]),
                "brow": np.ascontiguousarray(b[rows].reshape(1, G3)),
                "bhhn": np.ascontiguousarray(b_hh[2 * H + sl].reshape(1, HS)),
                "h0c": np.ascontiguousarray(hid[sl].reshape(1, HS)),
                "ones": ones,
                "ident": ident,
            }
        )
    return in_maps


_NC = None


def run(inputs, trace=False):
    global _NC
    if _NC is None:
        _NC = build_nc()
    res = run_bass_kernel_spmd(_NC, make_in_maps(inputs), list(range(N_CORES)), trace=trace)
    outputs = np.concatenate([res.results[c]["out"] for c in range(N_CORES)], axis=1)
    new_hidden = np.ascontiguousarray(outputs[STEPS - 1].reshape(1, 1, H))
    return (outputs, new_hidden), res


def kernel(**inputs):
    (outputs, new_hidden), _ = run(inputs)
    return outputs, new_hidden


# revision 5
# speedup vs baseline: 1.2201x; 1.2201x over previous
"""Trainium2 Bass kernel for nn_Encoder_57380763074770.

GRU-cell encoder over 64 independent "steps":
  xi  = concat(x[64,17], ip_emb[ip].reshape(64,8), port_emb[port].reshape(64,8))
  G   = xi @ W_ih.T + h0 @ W_hh.T + (b_ih + b_hh)       # [64, 384]
  r, z = sigmoid(G_r), sigmoid(G_z)
  n   = tanh(G_n + (r - 1) * hn),  hn = h0 @ W_hh_n.T + b_hh_n
  out = n + z * (h0 - n)                                # [64, 128]

Sharding: H=128 hidden columns split 8 ways -> each core owns 16 columns of
every gate (48 rows of W_ih/W_hh) and computes out[:, 16c:16c+16].

Per-core layout decisions (driven by HW profile):
- ALL small params ride in ONE packed [128, 403] f32 DMA (HWDGE ~0.6us fixed
  cost per dma_start dominates at these sizes); indices ride in a separate
  tiny [128, 5] i32 DMA that lands first so gathers start early.
- Embedding gathers run on-device via indirect DMA.  HW takes ONE index per
  partition, so 512 ip lookups = 4 gathers of 128 partitions and 128 port
  row lookups = 1 gather of 128 partitions, into a [128, 8] staging tile,
  then 2 strided SBUF->SBUF DMAs restore the [64, 16] row-major layout.
- PSUM accumulation ordered h-parts-first so only the last matmul waits on
  the gathered xi.
- Activation tables (sigmoid/tanh, ~1.3us each) pre-warmed with dummy ops
  so the loads overlap the gathers.
"""

import numpy as np

import concourse.bacc as bacc
import concourse.bass as bass
import concourse.mybir as mybir
import concourse.tile as tile
from concourse.bass_utils import run_bass_kernel_spmd

STEPS = 64
H = 128
N_CORES = 8
HS = H // N_CORES       # hidden cols per core = 16
G3 = 3 * HS             # gate rows per core = 48

F32 = mybir.dt.float32
I32 = mybir.dt.int32

# packed params column layout
C_WIH = 0               # [0:48, 0:33]    W_ih slice
C_WHH = 33              # [0:48, 33:161]  W_hh slice
C_ID = 161              # [0:64, 161:225] identity
C_XI = 225              # [0:64, 225:258] xi: x | ip-emb (gather dst) | port-emb
C_H = 258               # [0:128, 258]    h0 column
C_B = 259               # [0, 259:307]    b_ih + b_hh slice
C_BN = 307              # [0, 307:323]    b_hh n-gate slice
C_H0 = 323              # [0, 323:339]    h0 slice for this core
C_ONE = 339             # [0, 339:403]    ones row
F_PK = 403


def build_nc():
    nc = bacc.Bacc(None)

    idx_d = nc.declare_dram_parameter("idx", [H, 5], I32, isOutput=False)
    pk_d = nc.declare_dram_parameter("pk", [H, F_PK], F32, isOutput=False)
    ipe_d = nc.declare_dram_parameter("ip_emb", [256, 1], F32, isOutput=False)
    pte_d = nc.declare_dram_parameter("port_emb", [70000, 4], F32, isOutput=False)
    out_d = nc.declare_dram_parameter("out", [STEPS, HS], F32, isOutput=True)

    with tile.TileContext(nc) as tc:
        with (
            tc.tile_pool(name="sb", bufs=1) as sb,
            tc.tile_pool(name="ps", bufs=1, space="PSUM") as ps,
        ):
            idx = sb.tile([H, 5], I32)
            pk = sb.tile([H, F_PK], F32)
            st = sb.tile([H, 8], F32)
            wihT = sb.tile([33, G3], F32)
            whhT = sb.tile([H, G3], F32)
            xiT = sb.tile([33, STEPS], F32)
            warm = sb.tile([1, 2], F32)
            rz = sb.tile([STEPS, 2 * HS], F32)
            t2 = sb.tile([STEPS, HS], F32)
            u = sb.tile([STEPS, HS], F32)
            n = sb.tile([STEPS, HS], F32)
            d = sb.tile([STEPS, HS], F32)
            o = sb.tile([STEPS, HS], F32)

            nc.sync.dma_start(out=idx[:], in_=idx_d[:, :])
            nc.sync.dma_start(out=pk[:], in_=pk_d[:, :])

            # embedding gathers (gpsimd SWDGE, one index per partition)
            nc.gpsimd.indirect_dma_start(
                out=st[:, 4:8],
                out_offset=None,
                in_=pte_d[:, :],
                in_offset=bass.IndirectOffsetOnAxis(ap=idx[:, 4:5], axis=0),
            )
            for g in range(4):
                nc.gpsimd.indirect_dma_start(
                    out=st[:, g : g + 1],
                    out_offset=None,
                    in_=ipe_d[:, :],
                    in_offset=bass.IndirectOffsetOnAxis(ap=idx[:, g : g + 1], axis=0),
                )

            # pre-warm both activation tables while gathers run
            nc.scalar.activation(warm[:, 0:1], pk[0:1, C_ID : C_ID + 1],
                                 mybir.ActivationFunctionType.Tanh)
            nc.scalar.activation(warm[:, 1:2], pk[0:1, C_ID : C_ID + 1],
                                 mybir.ActivationFunctionType.Sigmoid)

            ident = pk[0:STEPS, C_ID : C_ID + STEPS]

            # weight transposes (PE) into contraction-major layout
            p_wihT = ps.tile([33, G3], F32, space="PSUM")
            nc.tensor.transpose(out=p_wihT[:], in_=pk[0:G3, C_WIH : C_WIH + 33],
                                identity=ident[:G3, :G3])
            nc.vector.tensor_copy(out=wihT[:], in_=p_wihT[:])

            p_whhT = ps.tile([H, G3], F32, space="PSUM")
            nc.tensor.transpose(out=p_whhT[:], in_=pk[0:G3, C_WHH : C_WHH + H],
                                identity=ident[:G3, :G3])
            nc.vector.tensor_copy(out=whhT[:], in_=p_whhT[:])

            hcol_b = pk[:, C_H : C_H + 1].to_broadcast([H, STEPS])
            ones = pk[0:1, C_ONE : C_ONE + STEPS]

            # h-dependent matmuls first; only the last G matmul waits on xi
            G = ps.tile([STEPS, G3], F32, space="PSUM")
            nc.tensor.matmul(out=G[:], lhsT=hcol_b, rhs=whhT[:], start=True, stop=False)
            nc.tensor.matmul(out=G[:], lhsT=ones, rhs=pk[0:1, C_B : C_B + G3],
                             start=False, stop=False)

            HN = ps.tile([STEPS, HS], F32, space="PSUM")
            nc.tensor.matmul(out=HN[:], lhsT=hcol_b, rhs=whhT[:, 2 * HS : 3 * HS],
                             start=True, stop=False)
            nc.tensor.matmul(out=HN[:], lhsT=ones, rhs=pk[0:1, C_BN : C_BN + HS],
                             start=False, stop=True)

            H0B = ps.tile([STEPS, HS], F32, space="PSUM")
            nc.tensor.matmul(out=H0B[:], lhsT=ones, rhs=pk[0:1, C_H0 : C_H0 + HS],
                             start=True, stop=True)

            # restore row-major layout of gathered embeddings into xi
            src = st[:, :].rearrange("p (b c) -> p b c", c=4)
            dst0 = pk[0:STEPS, C_XI + 17 : C_XI + 29].rearrange(
                "p (b c) -> p b c", c=4)[:, ::2, :]
            dst1 = pk[0:STEPS, C_XI + 21 : C_XI + 33].rearrange(
                "p (b c) -> p b c", c=4)[:, ::2, :]
            nc.sync.dma_start(out=dst0, in_=src[0:STEPS])
            nc.scalar.dma_start(out=dst1, in_=src[STEPS : 2 * STEPS])

            # xi transpose + final G accumulation
            p_xiT = ps.tile([33, STEPS], F32, space="PSUM")
            nc.tensor.transpose(out=p_xiT[:], in_=pk[0:STEPS, C_XI : C_XI + 33],
                                identity=ident)
            nc.vector.tensor_copy(out=xiT[:], in_=p_xiT[:])
            nc.tensor.matmul(out=G[:], lhsT=xiT[:], rhs=wihT[:], start=False, stop=True)

            # gates
            nc.scalar.activation(rz[:], G[:, 0 : 2 * HS],
                                 mybir.ActivationFunctionType.Sigmoid)
            r = rz[:, 0:HS]
            z = rz[:, HS : 2 * HS]
            A = mybir.AluOpType
            nc.vector.scalar_tensor_tensor(
                out=t2[:], in0=r, scalar=1.0, in1=HN[:], op0=A.subtract, op1=A.mult)
            nc.vector.tensor_add(out=u[:], in0=G[:, 2 * HS : 3 * HS], in1=t2[:])
            nc.scalar.activation(n[:], u[:], mybir.ActivationFunctionType.Tanh)
            nc.vector.scalar_tensor_tensor(
                out=d[:], in0=n[:], scalar=-1.0, in1=H0B[:], op0=A.mult, op1=A.add)
            nc.vector.tensor_mul(out=o[:], in0=z, in1=d[:])
            nc.vector.tensor_add(out=o[:], in0=n[:], in1=o[:])

            nc.sync.dma_start(out=out_d[:, :], in_=o[:])

    nc.finalize()
    return nc


def make_in_maps(inputs):
    x = np.asarray(inputs["x"], dtype=np.float32)
    ipi = np.asarray(inputs["ip"], dtype=np.int32)
    pti = np.asarray(inputs["port"], dtype=np.int32)
    hid = np.asarray(inputs["hidden"], dtype=np.float32).reshape(H)
    ip_emb = np.ascontiguousarray(np.asarray(inputs["ip_emb"], dtype=np.float32))
    port_emb = np.ascontiguousarray(np.asarray(inputs["port_emb"], dtype=np.float32))
    W_ih = np.asarray(inputs["W_ih"], dtype=np.float32)
    W_hh = np.asarray(inputs["W_hh"], dtype=np.float32)
    b = np.asarray(inputs["b_ih"], dtype=np.float32) + np.asarray(
        inputs["b_hh"], dtype=np.float32
    )
    b_hh = np.asarray(inputs["b_hh"], dtype=np.float32)

    idx = np.zeros((H, 5), dtype=np.int32)
    # ip gather g holds indices for columns k = 4j+g at partition j*64+s
    idx[:, 0:4] = ipi.reshape(STEPS, 2, 4).transpose(1, 0, 2).reshape(H, 4)
    idx[:, 4] = pti.T.reshape(H)

    in_maps = []
    for c in range(N_CORES):
        sl = np.arange(c * HS, (c + 1) * HS)
        rows = np.concatenate([sl, H + sl, 2 * H + sl])
        pk = np.zeros((H, F_PK), dtype=np.float32)
        pk[0:G3, C_WIH : C_WIH + 33] = W_ih[rows]
        pk[0:G3, C_WHH : C_WHH + H] = W_hh[rows]
        pk[0:STEPS, C_ID : C_ID + STEPS] = np.eye(STEPS, dtype=np.float32)
        pk[0:STEPS, C_XI : C_XI + 17] = x
        pk[:, C_H] = hid
        pk[0, C_B : C_B + G3] = b[rows]
        pk[0, C_BN : C_BN + HS] = b_hh[2 * H + sl]
        pk[0, C_H0 : C_H0 + HS] = hid[sl]
        pk[0, C_ONE : C_ONE + STEPS] = 1.0
        in_maps.append(
            {"idx": idx, "pk": pk, "ip_emb": ip_emb, "port_emb": port_emb}
        )
    return in_maps


_NC = None


def run(inputs, trace=False):
    global _NC
    if _NC is None:
        _NC = build_nc()
    res = run_bass_kernel_spmd(_NC, make_in_maps(inputs), list(range(N_CORES)), trace=trace)
    outputs = np.concatenate([res.results[c]["out"] for c in range(N_CORES)], axis=1)
    new_hidden = np.ascontiguousarray(outputs[STEPS - 1].reshape(1, 1, H))
    return (outputs, new_hidden), res


def kernel(**inputs):
    (outputs, new_hidden), _ = run(inputs)
    return outputs, new_hidden


# revision 7
# speedup vs baseline: 1.3317x; 1.0915x over previous
"""Trainium2 Bass kernel for nn_Encoder_57380763074770.

GRU-cell encoder over 64 independent "steps":
  xi  = concat(x[64,17], ip_emb[ip].reshape(64,8), port_emb[port].reshape(64,8))
  G   = xi @ W_ih.T + h0 @ W_hh.T + (b_ih + b_hh)       # [64, 384]
  r, z = sigmoid(G_r), sigmoid(G_z)
  n   = tanh(G_n + (r - 1) * hn),  hn = h0 @ W_hh_n.T + b_hh_n
  out = n + z * (h0 - n)                                # [64, 128]

Sharding: H=128 hidden columns split 8 ways -> each core owns 16 columns of
every gate (48 rows of W_ih/W_hh) and computes out[:, 16c:16c+16].

Layout decisions (driven by the HW profile):
- ALL small params ride in ONE packed [128, 403] f32 DMA; indices ride in a
  tiny [128, 5] i32 DMA that lands first so gathers start early (each
  dma_start costs ~0.6us issue + ~1.5us completion latency).
- Embedding gathers run on-device via indirect DMA (one index per
  partition): 512 ip lookups = 4 gathers of 128, 128 port-row lookups =
  1 gather of 128, into a [128, 8] staging tile.
- G is accumulated in PSUM as 7 matmuls: h-part and biases first, then one
  matmul per xi feature block with the staged gather tiles transposed
  straight on the PE ([128,4] -> [4,128]) -- no rearrange DMA hop.  Only
  the last two matmuls wait on the ip gathers.
- Activation tables (sigmoid/tanh, ~1.3us each) pre-warmed with dummy ops
  so the loads overlap the gathers.
"""

import numpy as np

import concourse.bacc as bacc
import concourse.bass as bass
import concourse.mybir as mybir
import concourse.tile as tile
from concourse.bass_utils import run_bass_kernel_spmd

STEPS = 64
H = 128
N_CORES = 8
HS = H // N_CORES       # hidden cols per core = 16
G3 = 3 * HS             # gate rows per core = 48

F32 = mybir.dt.float32
I32 = mybir.dt.int32

# packed params column layout
C_WIH = 0               # [0:48, 0:33]    W_ih slice (x | ip k=4j+g | port)
C_WHH = 33              # [0:48, 33:161]  W_hh slice
C_ID = 161              # [0:128, 161:289] identity (128 wide for PE transposes)
C_X = 289               # [0:64, 289:306] x
C_H = 306               # [0:128, 306]    h0 column
C_B = 307               # [0, 307:355]    b_ih + b_hh slice
C_BN = 355              # [0, 355:371]    b_hh n-gate slice
C_H0 = 371              # [0, 371:387]    h0 slice for this core
C_ONE = 387             # [0, 387:451]    ones row
F_PK = 451


def build_nc():
    nc = bacc.Bacc(None)

    idx_d = nc.declare_dram_parameter("idx", [H, 5], I32, isOutput=False)
    pk_d = nc.declare_dram_parameter("pk", [H, F_PK], F32, isOutput=False)
    ipe_d = nc.declare_dram_parameter("ip_emb", [256, 1], F32, isOutput=False)
    pte_d = nc.declare_dram_parameter("port_emb", [70000, 4], F32, isOutput=False)
    out_d = nc.declare_dram_parameter("out", [STEPS, HS], F32, isOutput=True)

    with tile.TileContext(nc) as tc:
        with (
            tc.tile_pool(name="sb", bufs=1) as sb,
            tc.tile_pool(name="ps", bufs=1, space="PSUM") as ps,
        ):
            idx = sb.tile([H, 5], I32)
            pk = sb.tile([H, F_PK], F32)
            st = sb.tile([H, 8], F32)
            warm = sb.tile([1, 2], F32)
            whhT = sb.tile([H, G3], F32)
            w_x = sb.tile([17, G3], F32)
            w_ip0 = sb.tile([4, G3], F32)
            w_ip1 = sb.tile([4, G3], F32)
            w_pt0 = sb.tile([4, G3], F32)
            w_pt1 = sb.tile([4, G3], F32)
            xT = sb.tile([17, STEPS], F32)
            ipT = sb.tile([4, H], F32)
            ptT = sb.tile([4, H], F32)
            rz = sb.tile([STEPS, 2 * HS], F32)
            t2 = sb.tile([STEPS, HS], F32)
            u = sb.tile([STEPS, HS], F32)
            n = sb.tile([STEPS, HS], F32)
            d = sb.tile([STEPS, HS], F32)
            o = sb.tile([STEPS, HS], F32)

            nc.sync.dma_start(out=idx[:], in_=idx_d[:, :])
            nc.sync.dma_start(out=pk[:], in_=pk_d[:, :])

            # embedding gathers (gpsimd SWDGE, one index per partition)
            nc.gpsimd.indirect_dma_start(
                out=st[:, 4:8],
                out_offset=None,
                in_=pte_d[:, :],
                in_offset=bass.IndirectOffsetOnAxis(ap=idx[:, 4:5], axis=0),
            )
            for g in range(4):
                nc.gpsimd.indirect_dma_start(
                    out=st[:, g : g + 1],
                    out_offset=None,
                    in_=ipe_d[:, :],
                    in_offset=bass.IndirectOffsetOnAxis(ap=idx[:, g : g + 1], axis=0),
                )

            # pre-warm both activation tables while gathers run
            nc.scalar.activation(warm[:, 0:1], pk[0:1, C_ID : C_ID + 1],
                                 mybir.ActivationFunctionType.Tanh)
            nc.scalar.activation(warm[:, 1:2], pk[0:1, C_ID : C_ID + 1],
                                 mybir.ActivationFunctionType.Sigmoid)

            ident = pk[:, C_ID : C_ID + H]
            id48 = ident[:G3, :G3]

            # weight transposes (PE) into contraction-major layout, early
            p_whhT = ps.tile([H, G3], F32, space="PSUM", tag="wt", bufs=2)
            nc.tensor.transpose(out=p_whhT[:], in_=pk[0:G3, C_WHH : C_WHH + H],
                                identity=id48)
            nc.vector.tensor_copy(out=whhT[:], in_=p_whhT[:])

            p_wx = ps.tile([17, G3], F32, space="PSUM", tag="wt", bufs=2)
            nc.tensor.transpose(out=p_wx[:], in_=pk[0:G3, C_WIH : C_WIH + 17],
                                identity=id48)
            nc.vector.tensor_copy(out=w_x[:], in_=p_wx[:])

            wblks = [(w_ip0, 17), (w_ip1, 21), (w_pt0, 25), (w_pt1, 29)]
            for wt, c0 in wblks:
                p_w = ps.tile([4, G3], F32, space="PSUM", name=f"p_w{c0}", tag="wt", bufs=2)
                nc.tensor.transpose(out=p_w[:], in_=pk[0:G3, C_WIH + c0 : C_WIH + c0 + 4],
                                    identity=id48)
                nc.vector.tensor_copy(out=wt[:], in_=p_w[:])

            # x transpose, early
            p_xT = ps.tile([17, STEPS], F32, space="PSUM", tag="wt", bufs=2)
            nc.tensor.transpose(out=p_xT[:], in_=pk[0:STEPS, C_X : C_X + 17],
                                identity=ident[:STEPS, :STEPS])
            nc.vector.tensor_copy(out=xT[:], in_=p_xT[:])

            hcol_b = pk[:, C_H : C_H + 1].to_broadcast([H, STEPS])
            ones = pk[0:1, C_ONE : C_ONE + STEPS]

            # h-dependent matmuls first
            HN = ps.tile([STEPS, HS], F32, space="PSUM")
            nc.tensor.matmul(out=HN[:], lhsT=hcol_b, rhs=whhT[:, 2 * HS : 3 * HS],
                             start=True, stop=False)
            nc.tensor.matmul(out=HN[:], lhsT=ones, rhs=pk[0:1, C_BN : C_BN + HS],
                             start=False, stop=True)

            H0B = ps.tile([STEPS, HS], F32, space="PSUM")
            nc.tensor.matmul(out=H0B[:], lhsT=ones, rhs=pk[0:1, C_H0 : C_H0 + HS],
                             start=True, stop=True)

            # port transpose (after the single port gather, early)
            p_ptT = ps.tile([4, H], F32, space="PSUM")
            nc.tensor.transpose(out=p_ptT[:], in_=st[:, 4:8], identity=ident)
            nc.vector.tensor_copy(out=ptT[:], in_=p_ptT[:])

            # G accumulation: everything not ip-dependent first
            G = ps.tile([STEPS, G3], F32, space="PSUM")
            nc.tensor.matmul(out=G[:], lhsT=hcol_b, rhs=whhT[:], start=True, stop=False)
            nc.tensor.matmul(out=G[:], lhsT=ones, rhs=pk[0:1, C_B : C_B + G3],
                             start=False, stop=False)
            nc.tensor.matmul(out=G[:], lhsT=xT[:], rhs=w_x[:], start=False, stop=False)
            nc.tensor.matmul(out=G[:], lhsT=ptT[:, 0:STEPS], rhs=w_pt0[:],
                             start=False, stop=False)
            nc.tensor.matmul(out=G[:], lhsT=ptT[:, STEPS : 2 * STEPS], rhs=w_pt1[:],
                             start=False, stop=False)

            # ip transpose (after all 4 ip gathers) + final G matmuls
            p_ipT = ps.tile([4, H], F32, space="PSUM")
            nc.tensor.transpose(out=p_ipT[:], in_=st[:, 0:4], identity=ident)
            nc.vector.tensor_copy(out=ipT[:], in_=p_ipT[:])
            nc.tensor.matmul(out=G[:], lhsT=ipT[:, 0:STEPS], rhs=w_ip0[:],
                             start=False, stop=False)
            nc.tensor.matmul(out=G[:], lhsT=ipT[:, STEPS : 2 * STEPS], rhs=w_ip1[:],
                             start=False, stop=True)

            # gates
            nc.scalar.activation(rz[:], G[:, 0 : 2 * HS],
                                 mybir.ActivationFunctionType.Sigmoid)
            r = rz[:, 0:HS]
            z = rz[:, HS : 2 * HS]
            A = mybir.AluOpType
            nc.vector.scalar_tensor_tensor(
                out=t2[:], in0=r, scalar=1.0, in1=HN[:], op0=A.subtract, op1=A.mult)
            nc.vector.tensor_add(out=u[:], in0=G[:, 2 * HS : 3 * HS], in1=t2[:])
            nc.scalar.activation(n[:], u[:], mybir.ActivationFunctionType.Tanh)
            nc.vector.scalar_tensor_tensor(
                out=d[:], in0=n[:], scalar=-1.0, in1=H0B[:], op0=A.mult, op1=A.add)
            nc.vector.tensor_mul(out=o[:], in0=z, in1=d[:])
            nc.vector.tensor_add(out=o[:], in0=n[:], in1=o[:])

            nc.sync.dma_start(out=out_d[:, :], in_=o[:])

    nc.finalize()
    return nc


def make_in_maps(inputs):
    x = np.asarray(inputs["x"], dtype=np.float32)
    ipi = np.asarray(inputs["ip"], dtype=np.int32)
    pti = np.asarray(inputs["port"], dtype=np.int32)
    hid = np.asarray(inputs["hidden"], dtype=np.float32).reshape(H)
    ip_emb = np.ascontiguousarray(np.asarray(inputs["ip_emb"], dtype=np.float32))
    port_emb = np.ascontiguousarray(np.asarray(inputs["port_emb"], dtype=np.float32))
    W_ih = np.asarray(inputs["W_ih"], dtype=np.float32)
    W_hh = np.asarray(inputs["W_hh"], dtype=np.float32)
    b = np.asarray(inputs["b_ih"], dtype=np.float32) + np.asarray(
        inputs["b_hh"], dtype=np.float32
    )
    b_hh = np.asarray(inputs["b_hh"], dtype=np.float32)

    idx = np.zeros((H, 5), dtype=np.int32)
    # ip gather g holds indices for column k = 4j+g at partition j*64+s
    idx[:, 0:4] = ipi.reshape(STEPS, 2, 4).transpose(1, 0, 2).reshape(H, 4)
    idx[:, 4] = pti.T.reshape(H)

    in_maps = []
    for c in range(N_CORES):
        sl = np.arange(c * HS, (c + 1) * HS)
        rows = np.concatenate([sl, H + sl, 2 * H + sl])
        pk = np.zeros((H, F_PK), dtype=np.float32)
        pk[0:G3, C_WIH : C_WIH + 33] = W_ih[rows]
        pk[0:G3, C_WHH : C_WHH + H] = W_hh[rows]
        pk[:, C_ID : C_ID + H] = np.eye(H, dtype=np.float32)
        pk[0:STEPS, C_X : C_X + 17] = x
        pk[:, C_H] = hid
        pk[0, C_B : C_B + G3] = b[rows]
        pk[0, C_BN : C_BN + HS] = b_hh[2 * H + sl]
        pk[0, C_H0 : C_H0 + HS] = hid[sl]
        pk[0, C_ONE : C_ONE + STEPS] = 1.0
        in_maps.append(
            {"idx": idx, "pk": pk, "ip_emb": ip_emb, "port_emb": port_emb}
        )
    return in_maps


_NC = None


def run(inputs, trace=False):
    global _NC
    if _NC is None:
        _NC = build_nc()
    res = run_bass_kernel_spmd(_NC, make_in_maps(inputs), list(range(N_CORES)), trace=trace)
    outputs = np.concatenate([res.results[c]["out"] for c in range(N_CORES)], axis=1)
    new_hidden = np.ascontiguousarray(outputs[STEPS - 1].reshape(1, 1, H))
    return (outputs, new_hidden), res


def kernel(**inputs):
    (outputs, new_hidden), _ = run(inputs)
    return outputs, new_hidden


# revision 8
# speedup vs baseline: 1.3896x; 1.0435x over previous
"""Trainium2 Bass kernel for nn_Encoder_57380763074770.

GRU-cell encoder over 64 independent "steps":
  xi  = concat(x[64,17], ip_emb[ip].reshape(64,8), port_emb[port].reshape(64,8))
  G   = xi @ W_ih.T + h0 @ W_hh.T + (b_ih + b_hh)       # [64, 384]
  r, z = sigmoid(G_r), sigmoid(G_z)
  n   = tanh(G_n + (r - 1) * hn),  hn = h0 @ W_hh_n.T + b_hh_n
  out = n + z * (h0 - n)                                # [64, 128]

Sharding: H=128 hidden columns split 8 ways -> each core owns 16 columns of
every gate (48 rows of W_ih/W_hh) and computes out[:, 16c:16c+16].

Layout decisions (driven by the HW profile -- fixed costs dominate at this
size: ~0.6us per dma_start issue, ~1.5-2us DMA completion latency, ~1.1us
per indirect DMA on the Q7, ~1.3us per activation-table load):
- Params ride in ONE packed [128, 451] f32 DMA on Sync; the DVE-gather
  blocks (iota, replicated 256-entry ip table, f32 ip indices) ride in a
  [64, 520] DMA issued on the Scalar-engine HWDGE in parallel; the port
  indices ride in a tiny [128, 1] DMA.
- The 512 ip_emb lookups (256-entry table) are computed on the VECTOR
  engine as one-hot compare + multiply + blocked reduce -- no indirect
  DMAs.  Only the port gather (70000x4 table) uses an indirect DMA:
  128 row lookups = 1 gather of 128 partitions.
- G is accumulated in PSUM with h-parts/biases/x first; only the last
  matmul waits on the DVE ip path.
- Activation tables pre-warmed with dummy ops so loads overlap the DMAs.
"""

import numpy as np

import concourse.bacc as bacc
import concourse.bass as bass
import concourse.mybir as mybir
import concourse.tile as tile
from concourse.bass_utils import run_bass_kernel_spmd

STEPS = 64
H = 128
N_CORES = 8
HS = H // N_CORES       # hidden cols per core = 16
G3 = 3 * HS             # gate rows per core = 48

F32 = mybir.dt.float32
I32 = mybir.dt.int32

# packed params column layout ("pk", [128, F_PK])
C_WIH = 0               # [0:48, 0:33]    W_ih slice (x | ip | port feature order)
C_WHH = 33              # [0:48, 33:161]  W_hh slice
C_ID = 161              # [0:128, 161:289] identity
C_X = 289               # [0:64, 289:306] x
C_H = 306               # [0:128, 306]    h0 column
C_B = 307               # [0, 307:355]    b_ih + b_hh slice
C_BN = 355              # [0, 355:371]    b_hh n-gate slice
C_H0 = 371              # [0, 371:387]    h0 slice for this core
C_ONE = 387             # [0, 387:451]    ones row
F_PK = 451

# DVE-gather block layout ("dv", [64, F_DV])
D_IOT = 0               # [0:64, 0:256]   iota row 0..255 per partition
D_EMB = 256             # [0:64, 256:512] ip_emb values replicated per row
D_IPF = 512             # [0:64, 512:520] ip indices as f32
F_DV = 520

V = 256                 # ip table size


def build_nc():
    nc = bacc.Bacc(None)

    idx_d = nc.declare_dram_parameter("idx", [H, 1], I32, isOutput=False)
    dv_d = nc.declare_dram_parameter("dv", [STEPS, F_DV], F32, isOutput=False)
    pk_d = nc.declare_dram_parameter("pk", [H, F_PK], F32, isOutput=False)
    pte_d = nc.declare_dram_parameter("port_emb", [70000, 4], F32, isOutput=False)
    out_d = nc.declare_dram_parameter("out", [STEPS, HS], F32, isOutput=True)

    with tile.TileContext(nc) as tc:
        with (
            tc.tile_pool(name="sb", bufs=1) as sb,
            tc.tile_pool(name="ps", bufs=1, space="PSUM") as ps,
        ):
            idx = sb.tile([H, 1], I32)
            dv = sb.tile([STEPS, F_DV], F32)
            pk = sb.tile([H, F_PK], F32)
            st = sb.tile([H, 4], F32)
            warm = sb.tile([1, 2], F32)
            eq = sb.tile([STEPS, 8 * V], F32)
            pr = sb.tile([STEPS, 8 * V], F32)
            ipf = sb.tile([STEPS, 8], F32)
            whhT = sb.tile([H, G3], F32)
            w_x = sb.tile([17, G3], F32)
            w_ip = sb.tile([8, G3], F32)
            w_pt0 = sb.tile([4, G3], F32)
            w_pt1 = sb.tile([4, G3], F32)
            xT = sb.tile([17, STEPS], F32)
            ipT = sb.tile([8, STEPS], F32)
            ptT = sb.tile([4, H], F32)
            rz = sb.tile([STEPS, 2 * HS], F32)
            t2 = sb.tile([STEPS, HS], F32)
            u = sb.tile([STEPS, HS], F32)
            n = sb.tile([STEPS, HS], F32)
            zz = sb.tile([STEPS, HS], F32)
            zh = sb.tile([STEPS, HS], F32)
            m = sb.tile([STEPS, HS], F32)
            o = sb.tile([STEPS, HS], F32)

            # DMAs: idx on Sync, dv on Scalar HWDGE (parallel), pk on Sync
            nc.sync.dma_start(out=idx[:], in_=idx_d[:, :])
            nc.scalar.dma_start(out=dv[:], in_=dv_d[:, :])
            nc.sync.dma_start(out=pk[:], in_=pk_d[:, :])

            # the single port gather (gpsimd SWDGE, one row index per partition)
            nc.gpsimd.indirect_dma_start(
                out=st[:],
                out_offset=None,
                in_=pte_d[:, :],
                in_offset=bass.IndirectOffsetOnAxis(ap=idx[:, :], axis=0),
            )

            # pre-warm both activation tables
            nc.scalar.activation(warm[:, 0:1], dv[0:1, 0:1],
                                 mybir.ActivationFunctionType.Tanh)
            nc.scalar.activation(warm[:, 1:2], dv[0:1, 0:1],
                                 mybir.ActivationFunctionType.Sigmoid)

            # ip embedding on DVE: one-hot compare, multiply, blocked reduce
            A = mybir.AluOpType
            ipb = dv[:, D_IPF : D_IPF + 8].unsqueeze(2).broadcast_to([STEPS, 8, V])
            iob = dv[:, D_IOT : D_IOT + V].unsqueeze(1).broadcast_to([STEPS, 8, V])
            emb = dv[:, D_EMB : D_EMB + V].unsqueeze(1).broadcast_to([STEPS, 8, V])
            eq3 = eq[:, :].rearrange("p (k v) -> p k v", v=V)
            pr3 = pr[:, :].rearrange("p (k v) -> p k v", v=V)
            nc.vector.tensor_tensor(out=eq3, in0=ipb, in1=iob, op=A.is_equal)
            nc.vector.tensor_tensor(out=pr3, in0=eq3, in1=emb, op=A.mult)
            nc.vector.tensor_reduce(out=ipf[:, :].unsqueeze(2), in_=pr3,
                                    axis=mybir.AxisListType.X, op=A.add)

            ident = pk[:, C_ID : C_ID + H]
            id48 = ident[:G3, :G3]

            # weight transposes (PE), copies on ACT
            p_whhT = ps.tile([H, G3], F32, space="PSUM", tag="wt", bufs=2)
            nc.tensor.transpose(out=p_whhT[:], in_=pk[0:G3, C_WHH : C_WHH + H],
                                identity=id48)
            nc.scalar.copy(out=whhT[:], in_=p_whhT[:])

            p_wx = ps.tile([17, G3], F32, space="PSUM", tag="wt", bufs=2)
            nc.tensor.transpose(out=p_wx[:], in_=pk[0:G3, C_WIH : C_WIH + 17],
                                identity=id48)
            nc.scalar.copy(out=w_x[:], in_=p_wx[:])

            p_wip = ps.tile([8, G3], F32, space="PSUM", tag="wt", bufs=2)
            nc.tensor.transpose(out=p_wip[:], in_=pk[0:G3, C_WIH + 17 : C_WIH + 25],
                                identity=id48)
            nc.scalar.copy(out=w_ip[:], in_=p_wip[:])

            p_wp0 = ps.tile([4, G3], F32, space="PSUM", tag="wt", bufs=2)
            nc.tensor.transpose(out=p_wp0[:], in_=pk[0:G3, C_WIH + 25 : C_WIH + 29],
                                identity=id48)
            nc.scalar.copy(out=w_pt0[:], in_=p_wp0[:])

            p_wp1 = ps.tile([4, G3], F32, space="PSUM", tag="wt", bufs=2)
            nc.tensor.transpose(out=p_wp1[:], in_=pk[0:G3, C_WIH + 29 : C_WIH + 33],
                                identity=id48)
            nc.scalar.copy(out=w_pt1[:], in_=p_wp1[:])

            # x transpose, early
            p_xT = ps.tile([17, STEPS], F32, space="PSUM", tag="wt", bufs=2)
            nc.tensor.transpose(out=p_xT[:], in_=pk[0:STEPS, C_X : C_X + 17],
                                identity=ident[:STEPS, :STEPS])
            nc.scalar.copy(out=xT[:], in_=p_xT[:])

            hcol_b = pk[:, C_H : C_H + 1].to_broadcast([H, STEPS])
            ones = pk[0:1, C_ONE : C_ONE + STEPS]

            # h-dependent matmuls
            HN = ps.tile([STEPS, HS], F32, space="PSUM")
            nc.tensor.matmul(out=HN[:], lhsT=hcol_b, rhs=whhT[:, 2 * HS : 3 * HS],
                             start=True, stop=False)
            nc.tensor.matmul(out=HN[:], lhsT=ones, rhs=pk[0:1, C_BN : C_BN + HS],
                             start=False, stop=True)

            H0B = ps.tile([STEPS, HS], F32, space="PSUM")
            nc.tensor.matmul(out=H0B[:], lhsT=ones, rhs=pk[0:1, C_H0 : C_H0 + HS],
                             start=True, stop=True)

            # port transpose (after the gather)
            p_ptT = ps.tile([4, H], F32, space="PSUM")
            nc.tensor.transpose(out=p_ptT[:], in_=st[:], identity=ident)
            nc.scalar.copy(out=ptT[:], in_=p_ptT[:])

            # G accumulation: everything not ip-dependent first
            G = ps.tile([STEPS, G3], F32, space="PSUM")
            nc.tensor.matmul(out=G[:], lhsT=hcol_b, rhs=whhT[:], start=True, stop=False)
            nc.tensor.matmul(out=G[:], lhsT=ones, rhs=pk[0:1, C_B : C_B + G3],
                             start=False, stop=False)
            nc.tensor.matmul(out=G[:], lhsT=xT[:], rhs=w_x[:], start=False, stop=False)
            nc.tensor.matmul(out=G[:], lhsT=ptT[:, 0:STEPS], rhs=w_pt0[:],
                             start=False, stop=False)
            nc.tensor.matmul(out=G[:], lhsT=ptT[:, STEPS : 2 * STEPS], rhs=w_pt1[:],
                             start=False, stop=False)

            # ip transpose + final G matmul
            p_ipT = ps.tile([8, STEPS], F32, space="PSUM")
            nc.tensor.transpose(out=p_ipT[:], in_=ipf[:, :],
                                identity=ident[:STEPS, :STEPS])
            nc.scalar.copy(out=ipT[:], in_=p_ipT[:])
            nc.tensor.matmul(out=G[:], lhsT=ipT[:], rhs=w_ip[:], start=False, stop=True)

            # gates
            nc.scalar.activation(rz[:], G[:, 0 : 2 * HS],
                                 mybir.ActivationFunctionType.Sigmoid)
            r = rz[:, 0:HS]
            z = rz[:, HS : 2 * HS]
            nc.vector.scalar_tensor_tensor(
                out=t2[:], in0=r, scalar=1.0, in1=HN[:], op0=A.subtract, op1=A.mult)
            nc.vector.tensor_add(out=u[:], in0=G[:, 2 * HS : 3 * HS], in1=t2[:])
            # overlap with tanh: zz = z-1, zh = z*h0
            nc.vector.tensor_scalar_add(out=zz[:], in0=z, scalar1=-1.0)
            nc.vector.tensor_mul(out=zh[:], in0=z, in1=H0B[:])
            nc.scalar.activation(n[:], u[:], mybir.ActivationFunctionType.Tanh)
            # o = z*h0 - n*(z-1) = n + z*(h0-n)
            nc.vector.tensor_mul(out=m[:], in0=n[:], in1=zz[:])
            nc.vector.tensor_sub(out=o[:], in0=zh[:], in1=m[:])

            nc.sync.dma_start(out=out_d[:, :], in_=o[:])

    nc.finalize()
    return nc


def make_in_maps(inputs):
    x = np.asarray(inputs["x"], dtype=np.float32)
    ipi = np.asarray(inputs["ip"], dtype=np.int32)
    pti = np.asarray(inputs["port"], dtype=np.int32)
    hid = np.asarray(inputs["hidden"], dtype=np.float32).reshape(H)
    ip_emb = np.ascontiguousarray(np.asarray(inputs["ip_emb"], dtype=np.float32))
    port_emb = np.ascontiguousarray(np.asarray(inputs["port_emb"], dtype=np.float32))
    W_ih = np.asarray(inputs["W_ih"], dtype=np.float32)
    W_hh = np.asarray(inputs["W_hh"], dtype=np.float32)
    b = np.asarray(inputs["b_ih"], dtype=np.float32) + np.asarray(
        inputs["b_hh"], dtype=np.float32
    )
    b_hh = np.asarray(inputs["b_hh"], dtype=np.float32)

    idx = np.ascontiguousarray(pti.T.reshape(H, 1))  # port[s,k] at partition k*64+s

    dv = np.zeros((STEPS, F_DV), dtype=np.float32)
    dv[:, D_IOT : D_IOT + V] = np.arange(V, dtype=np.float32)
    dv[:, D_EMB : D_EMB + V] = ip_emb[:, 0]
    dv[:, D_IPF : D_IPF + 8] = ipi.astype(np.float32)

    in_maps = []
    for c in range(N_CORES):
        sl = np.arange(c * HS, (c + 1) * HS)
        rows = np.concatenate([sl, H + sl, 2 * H + sl])
        pk = np.zeros((H, F_PK), dtype=np.float32)
        pk[0:G3, C_WIH : C_WIH + 33] = W_ih[rows]
        pk[0:G3, C_WHH : C_WHH + H] = W_hh[rows]
        pk[:, C_ID : C_ID + H] = np.eye(H, dtype=np.float32)
        pk[0:STEPS, C_X : C_X + 17] = x
        pk[:, C_H] = hid
        pk[0, C_B : C_B + G3] = b[rows]
        pk[0, C_BN : C_BN + HS] = b_hh[2 * H + sl]
        pk[0, C_H0 : C_H0 + HS] = hid[sl]
        pk[0, C_ONE : C_ONE + STEPS] = 1.0
        in_maps.append(
            {"idx": idx, "dv": dv, "pk": pk, "port_emb": port_emb}
        )
    return in_maps


_NC = None


def run(inputs, trace=False):
    global _NC
    if _NC is None:
        _NC = build_nc()
    res = run_bass_kernel_spmd(_NC, make_in_maps(inputs), list(range(N_CORES)), trace=trace)
    outputs = np.concatenate([res.results[c]["out"] for c in range(N_CORES)], axis=1)
    new_hidden = np.ascontiguousarray(outputs[STEPS - 1].reshape(1, 1, H))
    return (outputs, new_hidden), res


def kernel(**inputs):
    (outputs, new_hidden), _ = run(inputs)
    return outputs, new_hidden
